# revision 22
# baseline (speedup 1.0000x reference)
"""Trainium2 Bass kernel for nn_ClassicalSelfAttention (B=4, S=2048, E=1024).

Reference computation (fp32):
    w_qkv = rotation_params.reshape(3E, E); w_out = entangle_params.reshape(E, E)
    qkv = x @ w_qkv.T; q, k, v = split(qkv)
    scores = (q / sqrt(64)) @ k.T          # full-E attention, no heads
    attn = softmax(scores, axis=-1)
    out = (attn @ v) @ w_out.T
    result = sigmoid(out @ gate_w.T) * out

Algebraic folding (host-side, fp32):
    M  = Wq^T Wk / 8     -> scores = (x M) x^T       (kills the K projection)
    W2 = wo Wv           -> out    = (attn x) W2^T   (kills the V projection)
    W3 = gw wo Wv        -> gate_l = (attn x) W3^T   (decouples gate from out)

Sharding: 8 cores = 4 batches x 2 query-halves. Key order is rotated per
query-half so each core's queries are always columns 0:1024 of its xT input
(softmax and attn@x are permutation-invariant in key order).

All matmuls run in bf16 (full PE speed), fp32 PSUM accumulation. Softmax is
computed transposed (scoresT[kj, qi]) so no PE transpose of attn is needed:
    q'T[f, qi]    = M.T-blocks @ xT            (q' projection)
    scoresT[kj, qi] = xT[:,kj-block].T @ q'T   (stationary x, moving q')
    expT = exp(scoresT)     unnormalized, bf16 (scalar engine, psum -> sbuf)
    denom[1, qi] = ones[128,1].T @ expT        (PE, accumulated over kj tiles)
    ao_unT[e, qi] = xn[kj,e-block].T @ expT    (stationary xn, moving expT)
    rb[128, qi] = ones_row.T @ (1/denom)       (PE broadcast of reciprocal)
    aoT = ao_unT * rb       (normalization folded into psum->sbuf copy, DVE)
    outT = W2T-blocks @ aoT;  gateT = W3T-blocks @ aoT
    result^T = sigmoid(gateT) * outT           (bf16 out, DMA per f-tile)
Host untransposes the per-core [E, 1024] bf16 result tiles.
"""

from contextlib import ExitStack

import numpy as np
import ml_dtypes

import concourse.bass as bass
import concourse.tile as tile
from concourse import bacc, mybir
from concourse.bass_utils import run_bass_kernel_spmd

F32 = mybir.dt.float32
BF16 = mybir.dt.bfloat16
F8 = mybir.dt.float8e4
NPBF16 = ml_dtypes.bfloat16
NPF8 = ml_dtypes.float8_e4m3

W3_SCALE = 256.0   # host-folded into w3q quantization
AO_SCALE = 32.0    # folded into the fp8 copy of aoT
SG_SCALE = 1.0 / (W3_SCALE * AO_SCALE)

P = 128
E = 1024
B = 4
S = 2048
SK = S            # keys per core (full batch sequence)
SQ = S // 2       # queries per core (half)
ET = E // P       # 8 e-tiles
KT = SK // P      # 16 key tiles
NC = 512          # moving-operand chunk
QC = SQ // NC     # 2 query chunks
NCORES = 8


def _build_nc():
    nc = bacc.Bacc("TRN2", target_bir_lowering=False, debug=False,
                   num_devices=NCORES)
    xT = nc.dram_tensor("xT", [E, SK], BF16, kind="ExternalInput").ap()
    xn = nc.dram_tensor("xn", [SK, E], BF16, kind="ExternalInput").ap()
    m = nc.dram_tensor("m", [E, E], BF16, kind="ExternalInput").ap()
    w2T = nc.dram_tensor("w2T", [E, E], BF16, kind="ExternalInput").ap()
    w3q = nc.dram_tensor("w3q", [E // 2, 2 * E], F8, kind="ExternalInput").ap()
    outT = nc.dram_tensor("outT", [E, SQ], BF16, kind="ExternalOutput").ap()

    with tile.TileContext(nc) as tc, ExitStack() as ctx:
        _emit(tc, ctx, xT, xn, m, w2T, w3q, outT)
    nc.compile()
    return nc


def _emit(tc, ctx, xT, xn, m, w2T, w3q, outT):
    nc = tc.nc
    Exp = mybir.ActivationFunctionType.Exp
    Sigmoid = mybir.ActivationFunctionType.Sigmoid

    singles = ctx.enter_context(tc.tile_pool(name="singles", bufs=1))
    ones_col = singles.tile([P, 1], BF16, tag="ones_col")
    nc.vector.memset(ones_col[:], 1.0)
    ones_row = singles.tile([1, P], F32, tag="ones_row")
    nc.vector.memset(ones_row[:], 1.0)

    ps_mm = ctx.enter_context(tc.tile_pool(name="ps_mm", bufs=6, space="PSUM"))
    ps_d = ctx.enter_context(tc.tile_pool(name="ps_d", bufs=1, space="PSUM"))

    # long-lived inputs
    xt_pool = ctx.enter_context(tc.tile_pool(name="xt", bufs=1))
    xn_pool = ctx.enter_context(tc.tile_pool(name="xn", bufs=1))
    w_pool = ctx.enter_context(tc.tile_pool(name="wp", bufs=1))

    # staged lifetimes
    es_m = ExitStack()
    m_pool = es_m.enter_context(tc.tile_pool(name="mp", bufs=1))
    es_q = ExitStack()
    qt_pool = es_q.enter_context(tc.tile_pool(name="qt", bufs=1))
    es_exp = ExitStack()
    exp_pool = es_exp.enter_context(tc.tile_pool(name="exp", bufs=1, side="right"))

    # ---------------- DMA: phase-1 critical loads ----------------
    mt, xt = [], []
    for et in range(ET):
        tm = m_pool.tile([P, E], BF16, tag=f"m{et}", name=f"m{et}")
        nc.sync.dma_start(out=tm[:], in_=m[et * P:(et + 1) * P, :])
        mt.append(tm)
        t = xt_pool.tile([P, SK], BF16, tag=f"xt{et}", name=f"xt{et}")
        nc.sync.dma_start(out=t[:, 0:SQ], in_=xT[et * P:(et + 1) * P, 0:SQ])
        xt.append(t)

    # ---------------- Phase 1: q'T[f, qi] = M-blocks.T @ xT[:, 0:SQ] --------
    # Contraction is emitted in et-halves (0-3 then 4-7) per group of 3 ft so
    # the PE can start as soon as the first half of the M/xT DMAs lands
    # instead of stalling until all 4 MB arrive.
    qt = [qt_pool.tile([P, SQ], BF16, tag=f"q{i}", name=f"q{i}") for i in range(ET)]
    for fts in ((0, 1, 2), (3, 4, 5), (6, 7)):
        ps1 = {ft: [ps_mm.tile([P, NC], F32, tag="mm", name="mmp")
                    for _ in range(QC)] for ft in fts}
        for ep in range(4):
            for ft in fts:
                for et in (2 * ep, 2 * ep + 1):
                    for qc in range(QC):
                        nc.tensor.matmul(
                            ps1[ft][qc][:],
                            mt[et][:, ft * P:(ft + 1) * P],
                            xt[et][:, qc * NC:(qc + 1) * NC],
                            start=(et == 0), stop=(et == ET - 1),
                        )
        for ft in fts:
            for qc in range(QC):
                nc.vector.tensor_copy(
                    out=qt[ft][:, qc * NC:(qc + 1) * NC], in_=ps1[ft][qc][:])

    # ---------------- DMA: rest of the inputs (overlaps phases 1-3) --------
    for et in range(ET):
        nc.sync.dma_start(
            out=xt[et][:, SQ:SK], in_=xT[et * P:(et + 1) * P, SQ:SK])
    xnt = []
    for kt in range(KT):
        t = xn_pool.tile([P, E], BF16, tag=f"xn{kt}", name=f"xn{kt}")
        nc.sync.dma_start(out=t[:], in_=xn[kt * P:(kt + 1) * P, :])
        xnt.append(t)
    w2t, w3f8 = [], []
    for et in range(ET):
        t = w_pool.tile([P, E], BF16, tag=f"w2{et}", name=f"w2{et}")
        nc.sync.dma_start(out=t[:], in_=w2T[et * P:(et + 1) * P, :])
        w2t.append(t)
    for et2 in range(4):
        t = w_pool.tile([P, 2, E], F8, tag=f"w3{et2}", name=f"w3{et2}")
        for i in range(2):
            nc.sync.dma_start(
                out=t[:, i, :],
                in_=w3q[et2 * P:(et2 + 1) * P, i * E:(i + 1) * E])
        w3f8.append(t)

    # ---------------- Phase 2: scoresT -> exp -> denom ----------------
    # scoresT[kj, qi] = sum_e xT[e, kj] q'T[e, qi]; denom accumulated on PE
    # via ones-matmuls with a one-tile emission lag (exp of tile kt runs on
    # the scalar engine while PE does scores of tile kt+1).
    ext = [exp_pool.tile([P, SQ], BF16, tag=f"ex{i}", name=f"ex{i}")
           for i in range(KT)]
    psd = [ps_d.tile([1, NC], F32, tag=f"d{qc}", name=f"d{qc}")
           for qc in range(QC)]

    def emit_denom(kt):
        for qc in range(QC):
            nc.tensor.matmul(
                psd[qc][:],
                ones_col[:],
                ext[kt][:, qc * NC:(qc + 1) * NC],
                start=(kt == 0), stop=(kt == KT - 1),
            )

    for kt in range(KT):
        psums = [ps_mm.tile([P, NC], F32, tag="mm", name="mmp") for _ in range(QC)]
        for et in range(ET):
            for qc in range(QC):
                nc.tensor.matmul(
                    psums[qc][:],
                    xt[et][:, kt * P:(kt + 1) * P],
                    qt[et][:, qc * NC:(qc + 1) * NC],
                    start=(et == 0), stop=(et == ET - 1),
                )
        for qc in range(QC):
            nc.scalar.activation(
                out=ext[kt][:, qc * NC:(qc + 1) * NC],
                in_=psums[qc][:], func=Exp,
            )
        if kt > 0:
            emit_denom(kt - 1)
    # denom(KT-1) is emitted inside the first phase-3 group (below) so the
    # PE does not stall on the scalar engine's last exp at the boundary.

    es_q.close()   # qt freed
    es_m.close()   # M freed

    # ---------------- Phase 3: ao_unT -> normalized aoT ----------------
    ao_pool = ctx.enter_context(tc.tile_pool(name="ao", bufs=1))
    rb_pool = ctx.enter_context(tc.tile_pool(name="rb", bufs=1))
    aot = [ao_pool.tile([P, SQ], BF16, tag=f"ao{i}", name=f"ao{i}")
           for i in range(ET)]
    ao8 = [ao_pool.tile([P, 2, SQ], F8, tag=f"a8{i}", name=f"a8{i}")
           for i in range(4)]
    recip = rb_pool.tile([1, SQ], F32, tag="recip")
    rb = rb_pool.tile([P, SQ], F32, tag="rb")

    for et in range(ET):
        psums = [ps_mm.tile([P, NC], F32, tag="mm", name="mmp") for _ in range(QC)]
        for kt in range(KT):
            if et == 0 and kt == KT - 2:
                emit_denom(KT - 1)
            for qc in range(QC):
                nc.tensor.matmul(
                    psums[qc][:],
                    xnt[kt][:, et * P:(et + 1) * P],
                    ext[kt][:, qc * NC:(qc + 1) * NC],
                    start=(kt == 0), stop=(kt == KT - 1),
                )
        if et == 0:
            # reciprocal + PE broadcast of 1/denom to all 128 partitions;
            # runs on DVE/PE while the et=1 accumulation is in flight.
            for qc in range(QC):
                nc.vector.reciprocal(
                    out=recip[:, qc * NC:(qc + 1) * NC], in_=psd[qc][:])
            psb = [ps_mm.tile([P, NC], F32, tag="mm", name="mmp")
                   for _ in range(QC)]
            for qc in range(QC):
                nc.tensor.matmul(
                    psb[qc][:],
                    ones_row[:],
                    recip[:, qc * NC:(qc + 1) * NC],
                    start=True, stop=True,
                )
                nc.vector.tensor_copy(
                    out=rb[:, qc * NC:(qc + 1) * NC], in_=psb[qc][:])
        for qc in range(QC):
            nc.vector.tensor_mul(
                aot[et][:, qc * NC:(qc + 1) * NC],
                psums[qc][:],
                rb[:, qc * NC:(qc + 1) * NC],
            )
            # fp8 copy of aoT (scaled) for the DoubleRow gate matmul; the
            # Act engine converts dtypes in hardware and is idle here.
            nc.scalar.mul(
                out=ao8[et // 2][:, et % 2, qc * NC:(qc + 1) * NC],
                in_=aot[et][:, qc * NC:(qc + 1) * NC],
                mul=AO_SCALE,
            )

    es_exp.close()  # expT freed

    # ---------------- Phase 4: gateT (fp8 DoubleRow), outT, result ----------
    # Gate matmuls go first so the sigmoid overlaps the out-proj matmuls and
    # the post-last-matmul tail is just one DVE mul + DMA.
    DR = mybir.MatmulPerfMode.DoubleRow
    fin_pool = ctx.enter_context(tc.tile_pool(name="fin", bufs=2))
    for ft in range(ET):
        def emit_gate(ps_g):
            for et2 in range(4):
                for qc in range(QC):
                    nc.tensor.matmul(
                        ps_g[qc][:],
                        w3f8[et2][:, :, ft * P:(ft + 1) * P],
                        ao8[et2][:, :, qc * NC:(qc + 1) * NC],
                        start=(et2 == 0), stop=(et2 == 3),
                        perf_mode=DR,
                    )

        def emit_out(ps_o):
            for et in range(ET):
                for qc in range(QC):
                    nc.tensor.matmul(
                        ps_o[qc][:],
                        w2t[et][:, ft * P:(ft + 1) * P],
                        aot[et][:, qc * NC:(qc + 1) * NC],
                        start=(et == 0), stop=(et == ET - 1),
                    )

        ps_g = [ps_mm.tile([P, NC], F32, tag="mm", name="mmp") for _ in range(QC)]
        ps_o = [ps_mm.tile([P, NC], F32, tag="mm", name="mmp") for _ in range(QC)]
        if ft == 0:
            # ao8 (scalar engine) lags aot at the phase boundary: run the
            # bf16 out-proj first so the PE has work while ao8 lands.
            emit_out(ps_o)
            emit_gate(ps_g)
        else:
            emit_gate(ps_g)
            emit_out(ps_o)
        fin = fin_pool.tile([P, SQ], BF16, tag="fin")
        for qc in range(QC):
            sg = fin_pool.tile([P, NC], BF16, tag="sg")
            nc.scalar.activation(
                out=sg[:], in_=ps_g[qc][:], func=Sigmoid, scale=SG_SCALE)
            nc.vector.tensor_mul(
                fin[:, qc * NC:(qc + 1) * NC], ps_o[qc][:], sg[:])
        nc.sync.dma_start(out=outT[ft * P:(ft + 1) * P, :], in_=fin[:])


_NC_CACHE = None


def _get_nc():
    global _NC_CACHE
    if _NC_CACHE is None:
        _NC_CACHE = _build_nc()
    return _NC_CACHE


def _prep_in_maps(rotation_params, entangle_params, inputs, gate_w):
    w_qkv = np.asarray(rotation_params, dtype=np.float32).reshape(3 * E, E)
    wq, wk, wv = w_qkv[:E], w_qkv[E:2 * E], w_qkv[2 * E:]
    wo = np.asarray(entangle_params, dtype=np.float32).reshape(E, E)
    gw = np.asarray(gate_w, dtype=np.float32)
    x = np.asarray(inputs, dtype=np.float32)

    # host-folded weights (fp32 accuracy, then bf16 / fp8)
    m = ((wq.T @ wk) / 8.0).astype(NPBF16)          # [e, f]
    w2 = wo @ wv                                     # [f_out, e]
    w2T = np.ascontiguousarray(w2.T).astype(NPBF16)  # [e, f]
    # W3^T scaled and packed for DoubleRow: w3q[et2*128+p, i*E+f] =
    # 256 * W3T[et2*256 + i*128 + p, f], fp8e4 (TRN max 240).
    w3T = (gw @ w2).T * W3_SCALE
    w3q = np.ascontiguousarray(
        np.clip(w3T, -240.0, 240.0).reshape(4, 2, P, E)
        .transpose(0, 2, 1, 3).reshape(E // 2, 2 * E)).astype(NPF8)

    in_maps = []
    for c in range(NCORES):
        b, h = c // 2, c % 2
        xb = x[b]
        if h == 1:   # rotate keys so this core's queries sit at rows 0:SQ
            xb = np.concatenate([xb[SQ:], xb[:SQ]], axis=0)
        in_maps.append({
            "xT": np.ascontiguousarray(xb.T).astype(NPBF16),
            "xn": xb.astype(NPBF16),
            "m": m, "w2T": w2T, "w3q": w3q,
        })
    return in_maps


def _assemble(results):
    out = np.empty((B, S, E), dtype=np.float32)
    for c in range(NCORES):
        b, h = c // 2, c % 2
        out[b, h * SQ:(h + 1) * SQ, :] = results[c]["outT"].astype(np.float32).T
    return out


def _run(in_maps, trace=False):
    nc = _get_nc()
    return run_bass_kernel_spmd(nc, in_maps, core_ids=list(range(NCORES)),
                                trace=trace)


def kernel(rotation_params, entangle_params, inputs, gate_w):
    in_maps = _prep_in_maps(rotation_params, entangle_params, inputs, gate_w)
    res = _run(in_maps, trace=False)
    return _assemble(res.results)


# revision 26
# speedup vs baseline: 1.0354x; 1.0354x over previous
"""Trainium2 Bass kernel for nn_ClassicalSelfAttention (B=4, S=2048, E=1024).

Reference computation (fp32):
    w_qkv = rotation_params.reshape(3E, E); w_out = entangle_params.reshape(E, E)
    qkv = x @ w_qkv.T; q, k, v = split(qkv)
    scores = (q / sqrt(64)) @ k.T          # full-E attention, no heads
    attn = softmax(scores, axis=-1)
    out = (attn @ v) @ w_out.T
    result = sigmoid(out @ gate_w.T) * out

Algebraic folding (host-side, fp32):
    M  = Wq^T Wk / 8     -> scores = (x M) x^T       (kills the K projection)
    W2 = wo Wv           -> out    = (attn x) W2^T   (kills the V projection)
    W3 = gw wo Wv        -> gate_l = (attn x) W3^T   (decouples gate from out)

Sharding: 8 cores = 4 batches x 2 query-halves. Key order is rotated per
query-half so each core's queries are always columns 0:1024 of its xT input
(softmax and attn@x are permutation-invariant in key order).

All matmuls run in bf16 (full PE speed), fp32 PSUM accumulation. Softmax is
computed transposed (scoresT[kj, qi]) so no PE transpose of attn is needed:
    q'T[f, qi]    = M.T-blocks @ xT            (q' projection)
    scoresT[kj, qi] = xT[:,kj-block].T @ q'T   (stationary x, moving q')
    expT = exp(scoresT)     unnormalized, bf16 (scalar engine, psum -> sbuf)
    denom[1, qi] = ones[128,1].T @ expT        (PE, accumulated over kj tiles)
    ao_unT[e, qi] = xn[kj,e-block].T @ expT    (stationary xn, moving expT)
    rb[128, qi] = ones_row.T @ (1/denom)       (PE broadcast of reciprocal)
    aoT = ao_unT * rb       (normalization folded into psum->sbuf copy, DVE)
    outT = W2T-blocks @ aoT;  gateT = W3T-blocks @ aoT
    result^T = sigmoid(gateT) * outT           (bf16 out, DMA per f-tile)
Host untransposes the per-core [E, 1024] bf16 result tiles.
"""

from contextlib import ExitStack

import numpy as np
import ml_dtypes

import concourse.bass as bass
import concourse.tile as tile
from concourse import bacc, mybir
from concourse.bass_utils import run_bass_kernel_spmd

F32 = mybir.dt.float32
BF16 = mybir.dt.bfloat16
F8 = mybir.dt.float8e4
NPBF16 = ml_dtypes.bfloat16
NPF8 = ml_dtypes.float8_e4m3

W3_SCALE = 256.0   # host-folded into w3q quantization
AO_SCALE = 32.0    # folded into the fp8 copy of aoT
SG_SCALE = 1.0 / (W3_SCALE * AO_SCALE)

P = 128
E = 1024
B = 4
S = 2048
SK = S            # keys per core (full batch sequence)
SQ = S // 2       # queries per core (half)
ET = E // P       # 8 e-tiles
KT = SK // P      # 16 key tiles
NC = 512          # moving-operand chunk
QC = SQ // NC     # 2 query chunks
NCORES = 8


def _build_nc():
    nc = bacc.Bacc("TRN2", target_bir_lowering=False, debug=False,
                   num_devices=NCORES)
    xT = nc.dram_tensor("xT", [E, SK], BF16, kind="ExternalInput").ap()
    xn = nc.dram_tensor("xn", [SK, E], BF16, kind="ExternalInput").ap()
    m = nc.dram_tensor("m", [E, E], BF16, kind="ExternalInput").ap()
    w2T = nc.dram_tensor("w2T", [E, E], BF16, kind="ExternalInput").ap()
    w3q = nc.dram_tensor("w3q", [E // 2, 2 * E], F8, kind="ExternalInput").ap()
    outT = nc.dram_tensor("outT", [E, SQ], BF16, kind="ExternalOutput").ap()

    with tile.TileContext(nc) as tc, ExitStack() as ctx:
        _emit(tc, ctx, xT, xn, m, w2T, w3q, outT)
    nc.compile()
    return nc


def _emit(tc, ctx, xT, xn, m, w2T, w3q, outT):
    nc = tc.nc
    Exp = mybir.ActivationFunctionType.Exp
    Sigmoid = mybir.ActivationFunctionType.Sigmoid

    singles = ctx.enter_context(tc.tile_pool(name="singles", bufs=1))
    ones_col = singles.tile([P, 1], BF16, tag="ones_col")
    nc.vector.memset(ones_col[:], 1.0)
    ones_row = singles.tile([1, P], F32, tag="ones_row")
    nc.vector.memset(ones_row[:], 1.0)

    ps_mm = ctx.enter_context(tc.tile_pool(name="ps_mm", bufs=6, space="PSUM"))
    ps_d = ctx.enter_context(tc.tile_pool(name="ps_d", bufs=1, space="PSUM"))

    # long-lived inputs
    xt_pool = ctx.enter_context(tc.tile_pool(name="xt", bufs=1))
    xn_pool = ctx.enter_context(tc.tile_pool(name="xn", bufs=1))
    w_pool = ctx.enter_context(tc.tile_pool(name="wp", bufs=1))

    # staged lifetimes
    es_m = ExitStack()
    m_pool = es_m.enter_context(tc.tile_pool(name="mp", bufs=1))
    es_q = ExitStack()
    qt_pool = es_q.enter_context(tc.tile_pool(name="qt", bufs=1))
    es_exp = ExitStack()
    exp_pool = es_exp.enter_context(tc.tile_pool(name="exp", bufs=1, side="right"))

    # ---------------- DMA: phase-1 critical loads ----------------
    mt, xt = [], []
    for et in range(ET):
        tm = m_pool.tile([P, E], BF16, tag=f"m{et}", name=f"m{et}")
        nc.sync.dma_start(out=tm[:], in_=m[et * P:(et + 1) * P, :])
        mt.append(tm)
        t = xt_pool.tile([P, SK], BF16, tag=f"xt{et}", name=f"xt{et}")
        nc.sync.dma_start(out=t[:, 0:SQ], in_=xT[et * P:(et + 1) * P, 0:SQ])
        xt.append(t)

    # ---------------- Phase 1: q'T[f, qi] = M-blocks.T @ xT[:, 0:SQ] --------
    # Contraction is emitted in et-halves (0-3 then 4-7) per group of 3 ft so
    # the PE can start as soon as the first half of the M/xT DMAs lands
    # instead of stalling until all 4 MB arrive.
    qt = [qt_pool.tile([P, SQ], BF16, tag=f"q{i}", name=f"q{i}") for i in range(ET)]
    for fts in ((0, 1, 2), (3, 4, 5), (6, 7)):
        ps1 = {ft: [ps_mm.tile([P, NC], F32, tag="mm", name="mmp")
                    for _ in range(QC)] for ft in fts}
        for eh in range(2):
            for ft in fts:
                for et in range(eh * 4, eh * 4 + 4):
                    for qc in range(QC):
                        nc.tensor.matmul(
                            ps1[ft][qc][:],
                            mt[et][:, ft * P:(ft + 1) * P],
                            xt[et][:, qc * NC:(qc + 1) * NC],
                            start=(et == 0), stop=(et == ET - 1),
                        )
        for ft in fts:
            for qc in range(QC):
                nc.vector.tensor_copy(
                    out=qt[ft][:, qc * NC:(qc + 1) * NC], in_=ps1[ft][qc][:])

    # ---------------- DMA: rest of the inputs (overlaps phases 1-3) --------
    for et in range(ET):
        nc.sync.dma_start(
            out=xt[et][:, SQ:SK], in_=xT[et * P:(et + 1) * P, SQ:SK])
    xnt = []
    for kt in range(KT):
        t = xn_pool.tile([P, E], BF16, tag=f"xn{kt}", name=f"xn{kt}")
        nc.sync.dma_start(out=t[:], in_=xn[kt * P:(kt + 1) * P, :])
        xnt.append(t)
    w2t, w3f8 = [], []
    for et in range(ET):
        t = w_pool.tile([P, E], BF16, tag=f"w2{et}", name=f"w2{et}")
        nc.sync.dma_start(out=t[:], in_=w2T[et * P:(et + 1) * P, :])
        w2t.append(t)
    for et2 in range(4):
        t = w_pool.tile([P, 2, E], F8, tag=f"w3{et2}", name=f"w3{et2}")
        for i in range(2):
            nc.sync.dma_start(
                out=t[:, i, :],
                in_=w3q[et2 * P:(et2 + 1) * P, i * E:(i + 1) * E])
        w3f8.append(t)

    # ---------------- Phase 2: scoresT -> exp -> denom ----------------
    # scoresT[kj, qi] = sum_e xT[e, kj] q'T[e, qi]; denom accumulated on PE
    # via ones-matmuls with a one-tile emission lag (exp of tile kt runs on
    # the scalar engine while PE does scores of tile kt+1).
    ext = [exp_pool.tile([P, SQ], BF16, tag=f"ex{i}", name=f"ex{i}")
           for i in range(KT)]
    psd = [ps_d.tile([1, NC], F32, tag=f"d{qc}", name=f"d{qc}")
           for qc in range(QC)]

    def emit_denom(kt):
        for qc in range(QC):
            nc.tensor.matmul(
                psd[qc][:],
                ones_col[:],
                ext[kt][:, qc * NC:(qc + 1) * NC],
                start=(kt == 0), stop=(kt == KT - 1),
            )

    for kt in range(KT):
        psums = [ps_mm.tile([P, NC], F32, tag="mm", name="mmp") for _ in range(QC)]
        for et in range(ET):
            for qc in range(QC):
                nc.tensor.matmul(
                    psums[qc][:],
                    xt[et][:, kt * P:(kt + 1) * P],
                    qt[et][:, qc * NC:(qc + 1) * NC],
                    start=(et == 0), stop=(et == ET - 1),
                )
        for qc in range(QC):
            nc.scalar.activation(
                out=ext[kt][:, qc * NC:(qc + 1) * NC],
                in_=psums[qc][:], func=Exp,
            )
        if kt > 0:
            emit_denom(kt - 1)
    emit_denom(KT - 1)

    es_q.close()   # qt freed
    es_m.close()   # M freed

    # ---------------- Phase 3: ao_unT -> normalized aoT ----------------
    ao_pool = ctx.enter_context(tc.tile_pool(name="ao", bufs=1))
    rb_pool = ctx.enter_context(tc.tile_pool(name="rb", bufs=1))
    aot = [ao_pool.tile([P, SQ], BF16, tag=f"ao{i}", name=f"ao{i}")
           for i in range(ET)]
    ao8 = [ao_pool.tile([P, 2, SQ], F8, tag=f"a8{i}", name=f"a8{i}")
           for i in range(4)]
    recip = rb_pool.tile([1, SQ], F32, tag="recip")
    rb = rb_pool.tile([P, SQ], F32, tag="rb")

    for et in range(ET):
        psums = [ps_mm.tile([P, NC], F32, tag="mm", name="mmp") for _ in range(QC)]
        for kt in range(KT):
            for qc in range(QC):
                nc.tensor.matmul(
                    psums[qc][:],
                    xnt[kt][:, et * P:(et + 1) * P],
                    ext[kt][:, qc * NC:(qc + 1) * NC],
                    start=(kt == 0), stop=(kt == KT - 1),
                )
        if et == 0:
            # reciprocal + PE broadcast of 1/denom to all 128 partitions;
            # runs on DVE/PE while the et=1 accumulation is in flight.
            for qc in range(QC):
                nc.vector.reciprocal(
                    out=recip[:, qc * NC:(qc + 1) * NC], in_=psd[qc][:])
            psb = [ps_mm.tile([P, NC], F32, tag="mm", name="mmp")
                   for _ in range(QC)]
            for qc in range(QC):
                nc.tensor.matmul(
                    psb[qc][:],
                    ones_row[:],
                    recip[:, qc * NC:(qc + 1) * NC],
                    start=True, stop=True,
                )
                nc.vector.tensor_copy(
                    out=rb[:, qc * NC:(qc + 1) * NC], in_=psb[qc][:])
        for qc in range(QC):
            nc.vector.tensor_mul(
                aot[et][:, qc * NC:(qc + 1) * NC],
                psums[qc][:],
                rb[:, qc * NC:(qc + 1) * NC],
            )
            # fp8 copy of aoT (scaled) for the DoubleRow gate matmul; the
            # Act engine converts dtypes in hardware and is idle here.
            nc.scalar.mul(
                out=ao8[et // 2][:, et % 2, qc * NC:(qc + 1) * NC],
                in_=aot[et][:, qc * NC:(qc + 1) * NC],
                mul=AO_SCALE,
            )

    es_exp.close()  # expT freed

    # ---------------- Phase 4: gateT (fp8 DoubleRow), outT, result ----------
    # Gate matmuls go first so the sigmoid overlaps the out-proj matmuls and
    # the post-last-matmul tail is just one DVE mul + DMA.
    DR = mybir.MatmulPerfMode.DoubleRow
    fin_pool = ctx.enter_context(tc.tile_pool(name="fin", bufs=2))
    for ft in range(ET):
        def emit_gate(ps_g):
            for et2 in range(4):
                for qc in range(QC):
                    nc.tensor.matmul(
                        ps_g[qc][:],
                        w3f8[et2][:, :, ft * P:(ft + 1) * P],
                        ao8[et2][:, :, qc * NC:(qc + 1) * NC],
                        start=(et2 == 0), stop=(et2 == 3),
                        perf_mode=DR,
                    )

        def emit_out(ps_o):
            for et in range(ET):
                for qc in range(QC):
                    nc.tensor.matmul(
                        ps_o[qc][:],
                        w2t[et][:, ft * P:(ft + 1) * P],
                        aot[et][:, qc * NC:(qc + 1) * NC],
                        start=(et == 0), stop=(et == ET - 1),
                    )

        ps_g = [ps_mm.tile([P, NC], F32, tag="mm", name="mmp") for _ in range(QC)]
        emit_gate(ps_g)
        ps_o = [ps_mm.tile([P, NC], F32, tag="mm", name="mmp") for _ in range(QC)]
        emit_out(ps_o)
        fin = fin_pool.tile([P, SQ], BF16, tag="fin")
        for qc in range(QC):
            sg = fin_pool.tile([P, NC], BF16, tag="sg")
            nc.scalar.activation(
                out=sg[:], in_=ps_g[qc][:], func=Sigmoid, scale=SG_SCALE)
            nc.vector.tensor_mul(
                fin[:, qc * NC:(qc + 1) * NC], ps_o[qc][:], sg[:])
        nc.sync.dma_start(out=outT[ft * P:(ft + 1) * P, :], in_=fin[:])


_NC_CACHE = None


def _get_nc():
    global _NC_CACHE
    if _NC_CACHE is None:
        _NC_CACHE = _build_nc()
    return _NC_CACHE


def _prep_in_maps(rotation_params, entangle_params, inputs, gate_w):
    w_qkv = np.asarray(rotation_params, dtype=np.float32).reshape(3 * E, E)
    wq, wk, wv = w_qkv[:E], w_qkv[E:2 * E], w_qkv[2 * E:]
    wo = np.asarray(entangle_params, dtype=np.float32).reshape(E, E)
    gw = np.asarray(gate_w, dtype=np.float32)
    x = np.asarray(inputs, dtype=np.float32)

    # host-folded weights (fp32 accuracy, then bf16 / fp8)
    m = ((wq.T @ wk) / 8.0).astype(NPBF16)          # [e, f]
    w2 = wo @ wv                                     # [f_out, e]
    w2T = np.ascontiguousarray(w2.T).astype(NPBF16)  # [e, f]
    # W3^T scaled and packed for DoubleRow: w3q[et2*128+p, i*E+f] =
    # 256 * W3T[et2*256 + i*128 + p, f], fp8e4 (TRN max 240).
    w3T = (gw @ w2).T * W3_SCALE
    w3q = np.ascontiguousarray(
        np.clip(w3T, -240.0, 240.0).reshape(4, 2, P, E)
        .transpose(0, 2, 1, 3).reshape(E // 2, 2 * E)).astype(NPF8)

    in_maps = []
    for c in range(NCORES):
        b, h = c // 2, c % 2
        xb = x[b]
        if h == 1:   # rotate keys so this core's queries sit at rows 0:SQ
            xb = np.concatenate([xb[SQ:], xb[:SQ]], axis=0)
        in_maps.append({
            "xT": np.ascontiguousarray(xb.T).astype(NPBF16),
            "xn": xb.astype(NPBF16),
            "m": m, "w2T": w2T, "w3q": w3q,
        })
    return in_maps


def _assemble(results):
    out = np.empty((B, S, E), dtype=np.float32)
    for c in range(NCORES):
        b, h = c // 2, c % 2
        out[b, h * SQ:(h + 1) * SQ, :] = results[c]["outT"].astype(np.float32).T
    return out


def _run(in_maps, trace=False):
    nc = _get_nc()
    return run_bass_kernel_spmd(nc, in_maps, core_ids=list(range(NCORES)),
                                trace=trace)


def kernel(rotation_params, entangle_params, inputs, gate_w):
    in_maps = _prep_in_maps(rotation_params, entangle_params, inputs, gate_w)
    res = _run(in_maps, trace=False)
    return _assemble(res.results)
